# revision 8
# baseline (speedup 1.0000x reference)
"""AutoInformer encoder layer on 8 TRN2 NeuronCores.

Strategy
--------
Data-parallel over batch: core b handles x[b] (T=2048, C=1024).

Per-core math (all equivalent to the reference, FFT realized as dense
DFT matmuls):
  xT   = x.T                                       [c, t]   (PE transpose)
  q|k  = xT.T @ Wq|Wk  (no bias; DC-fixed later)   [t, c']  (f32r matmuls)
  Fqr  = Cos.T @ q ; Gi = Sin.T @ q  (= -fqi)      [f, c']
  Fkr  = Cos.T @ k ; Ki = Sin.T @ k  (= -fki)
  Fqr[0] += T*bq ; Fkr[0] += T*bk                  (bias only affects DC)
  Pr   = Fqr*Fkr + Gi*Ki ;  Pi = Fqr*Ki - Gi*Fkr   (elementwise)
  corrT= sum_f Pr[f,c]*Wc[f,tau] + Pi[f,c]*WsN[f,tau]   [c, tau]
  w    = softmax(corrT, axis=tau)  (free-axis reduce + ACT exp w/ accum)
  vsum = x @ rowsum(Wv) + sum(bv)                  [1, t]
  y1T  = w * vsum + xT                             [c, t] -> DRAM
  y1   = transpose(y1T)                            [t, c]
  x1   = LN(y1)        (free-axis stats)
  h1   = relu(W1.T @ x1T + bf1)                    [c', t] (bf16)
  h2   = h1.T @ W2 + bf2                           [t, c]
  out  = LN(x1 + h2)                               [t, c]
"""

import numpy as np

T = 2048
C = 1024
F = T // 2 + 1          # 1025 rfft bins
NT = T // 128           # 16 t tiles
NC = C // 128           # 8 c tiles
NF = 9                  # f tiles: 8 full + 1 (Nyquist row)
CHW = 256               # channel chunk width for the correlation path
NCHUNK = C // CHW       # 4
N_CORES = 8

_cache = {}


def _host_constants(inputs):
    import ml_dtypes

    t = np.arange(T, dtype=np.float64)
    f = np.arange(F, dtype=np.float64)
    ang = 2.0 * np.pi * np.outer(t, f) / T
    Cos = np.cos(ang).astype(np.float32)                     # [T, F]
    Sin = np.sin(ang).astype(np.float32)                     # [T, F]
    alpha = np.full(F, 2.0 / T, np.float64)
    alpha[0] = 1.0 / T
    alpha[-1] = 1.0 / T
    angi = 2.0 * np.pi * np.outer(f, t) / T
    Wc = (np.cos(angi) * alpha[:, None]).astype(np.float32)  # [F, T]
    WsN = (-np.sin(angi) * alpha[:, None]).astype(np.float32)

    bf16 = ml_dtypes.bfloat16
    consts = {
        "Cos": Cos,
        "Sin": Sin,
        "Wc": Wc,
        "WsN": WsN,
        "Wq": np.ascontiguousarray(inputs["Wq"], np.float32),
        "Wk": np.ascontiguousarray(inputs["Wk"], np.float32),
        "W1": np.ascontiguousarray(inputs["W1"]).astype(bf16),
        "W2": np.ascontiguousarray(inputs["W2"]).astype(bf16),
        "wvs": np.ascontiguousarray(inputs["Wv"].sum(axis=1, dtype=np.float64)
                                    .astype(np.float32).reshape(C, 1)),
        "Tbq": (T * inputs["bq"]).astype(np.float32).reshape(1, C),
        "Tbk": (T * inputs["bk"]).astype(np.float32).reshape(1, C),
        "bf1T": np.ascontiguousarray(inputs["bf1"].reshape(NC, 128).T
                                     .astype(np.float32)),      # [128, 8]
        "bf2row": inputs["bf2"].astype(bf16).reshape(1, C),
        "id32": np.eye(128, dtype=np.float32),
        "id16": np.eye(128, dtype=bf16),
        "ones32": np.ones((1, 128), np.float32),
        "ones16": np.ones((1, 128), bf16),
    }
    scalars = {
        "bvs": float(np.sum(inputs["bv"], dtype=np.float64)),
        "g1_trivial": bool(np.all(inputs["g1"] == 1.0) and np.all(inputs["be1"] == 0.0)),
        "g2_trivial": bool(np.all(inputs["g2"] == 1.0) and np.all(inputs["be2"] == 0.0)),
    }
    consts["g1row"] = inputs["g1"].astype(bf16).reshape(1, C)
    consts["be1row"] = inputs["be1"].astype(bf16).reshape(1, C)
    consts["g2row"] = inputs["g2"].astype(bf16).reshape(1, C)
    consts["be2row"] = inputs["be2"].astype(bf16).reshape(1, C)
    return consts, scalars


def _build(scalars):
    import concourse.bacc as bacc
    import concourse.bass as bass
    import concourse.mybir as mybir
    from concourse.tile import TileContext
    from concourse.alu_op_type import AluOpType

    f32 = mybir.dt.float32
    f32r = mybir.dt.float32r
    bf = mybir.dt.bfloat16
    AX = mybir.AxisListType.X
    ACT = mybir.ActivationFunctionType
    PSUM = bass.MemorySpace.PSUM

    nc = bacc.Bacc(None, target_bir_lowering=False)

    def dparam(name, shape, dt):
        return nc.declare_dram_parameter(name, list(shape), dt, isOutput=False)

    x_in = dparam("x", (T, C), f32)
    CosD = dparam("Cos", (T, F), f32r)
    SinD = dparam("Sin", (T, F), f32r)
    WcD = dparam("Wc", (F, T), f32r)
    WsND = dparam("WsN", (F, T), f32r)
    WqD = dparam("Wq", (C, C), f32r)
    WkD = dparam("Wk", (C, C), f32r)
    W1D = dparam("W1", (C, C), bf)
    W2D = dparam("W2", (C, C), bf)
    wvsD = dparam("wvs", (C, 1), f32r)
    TbqD = dparam("Tbq", (1, C), f32)
    TbkD = dparam("Tbk", (1, C), f32)
    bf1TD = dparam("bf1T", (128, NC), f32)
    bf2rowD = dparam("bf2row", (1, C), bf)
    id32D = dparam("id32", (128, 128), f32)
    id16D = dparam("id16", (128, 128), bf)
    ones32D = dparam("ones32", (1, 128), f32)
    ones16D = dparam("ones16", (1, 128), bf)
    g1rowD = dparam("g1row", (1, C), bf)
    be1rowD = dparam("be1row", (1, C), bf)
    g2rowD = dparam("g2row", (1, C), bf)
    be2rowD = dparam("be2row", (1, C), bf)
    out_d = nc.declare_dram_parameter("out", [T, C], f32, isOutput=True)

    y1T_d = nc.dram_tensor("y1Tstage", [C, T], bf)

    def r(ap):
        return ap.bitcast(f32r)

    with TileContext(nc) as tc:
        with tc.tile_pool(name="persist", bufs=1) as pP:
            # --- small persistent constants ---
            id32 = pP.tile([128, 128], f32, tag="id32")
            nc.sync.dma_start(id32[:], id32D[:])
            ones32 = pP.tile([1, 128], f32, tag="ones32")
            nc.sync.dma_start(ones32[:], ones32D[:])
            if not (scalars["g1_trivial"] and scalars["g2_trivial"]):
                ones16 = pP.tile([1, 128], bf, tag="ones16")
                nc.sync.dma_start(ones16[:], ones16D[:])
            wvs_sb = pP.tile([128, NC], f32r, tag="wvs")
            nc.sync.dma_start(wvs_sb.rearrange("p (k o) -> p k o", k=NC),
                              wvsD.rearrange("(k p) o -> p k o", p=128))
            Tbq_sb = pP.tile([1, C], f32, tag="Tbq")
            nc.sync.dma_start(Tbq_sb[:], TbqD[:])
            Tbk_sb = pP.tile([1, C], f32, tag="Tbk")
            nc.sync.dma_start(Tbk_sb[:], TbkD[:])

            # LN affine broadcast tiles, only when nontrivial
            gb = {}
            for trivial, gD, beD, key in (
                (scalars["g1_trivial"], g1rowD, be1rowD, 1),
                (scalars["g2_trivial"], g2rowD, be2rowD, 2),
            ):
                if trivial:
                    continue
                grow = pP.tile([1, C], bf, tag=f"g{key}row")
                nc.sync.dma_start(grow[:], gD[:])
                berow = pP.tile([1, C], bf, tag=f"be{key}row")
                nc.sync.dma_start(berow[:], beD[:])
                gbt = pP.tile([128, C], bf, tag=f"g{key}b")
                beb = pP.tile([128, C], bf, tag=f"be{key}b")
                with tc.tile_pool(name=f"psg{key}", bufs=2, space=PSUM) as psg:
                    for n in range(C // 512):
                        ps = psg.tile([128, 512], f32, tag="b")
                        nc.tensor.matmul(ps[:], ones16[:], grow[:, n * 512:(n + 1) * 512],
                                         start=True, stop=True)
                        nc.vector.tensor_copy(gbt[:, n * 512:(n + 1) * 512], ps[:])
                        ps2 = psg.tile([128, 512], f32, tag="b")
                        nc.tensor.matmul(ps2[:], ones16[:], berow[:, n * 512:(n + 1) * 512],
                                         start=True, stop=True)
                        nc.vector.tensor_copy(beb[:, n * 512:(n + 1) * 512], ps2[:])
                gb[key] = (gbt, beb)

            # --- phase 1: load x, build xT [c, t] ---
            xT = [pP.tile([128, T], f32r, tag=f"xT{i}", name=f"xT{i}") for i in range(NC)]
            with (
                tc.tile_pool(name="xload", bufs=3) as pX,
                tc.tile_pool(name="ps1", bufs=4, space=PSUM) as ps1,
            ):
                for tt in range(NT):
                    xt = pX.tile([128, C], f32, tag="x")
                    nc.sync.dma_start(xt[:], x_in[tt * 128:(tt + 1) * 128, :])
                    for cc in range(NC):
                        pt = ps1.tile([128, 128], f32, tag="tp")
                        nc.tensor.transpose(pt[:], xt[:, cc * 128:(cc + 1) * 128], id32[:])
                        nc.vector.tensor_copy(xT[cc][:, tt * 128:(tt + 1) * 128], pt[:])

            # --- vsum row + broadcast Vb ---
            vsrow = pP.tile([1, T], f32, tag="vsrow")
            Vb = pP.tile([128, T], bf, tag="Vb")
            with tc.tile_pool(name="psv", bufs=2, space=PSUM) as psv:
                for n in range(T // 512):
                    ps = psv.tile([1, 512], f32, tag="v")
                    for k in range(NC):
                        nc.tensor.matmul(ps[:], wvs_sb[:, k:k + 1],
                                         xT[k][:, n * 512:(n + 1) * 512],
                                         start=(k == 0), stop=(k == NC - 1))
                    nc.vector.tensor_scalar_add(vsrow[:, n * 512:(n + 1) * 512], ps[:],
                                                scalars["bvs"])
                for n in range(T // 512):
                    ps = psv.tile([128, 512], f32, tag="vb")
                    nc.tensor.matmul(ps[:], ones32[:], vsrow[:, n * 512:(n + 1) * 512],
                                     start=True, stop=True)
                    nc.vector.tensor_copy(Vb[:, n * 512:(n + 1) * 512], ps[:])

            # --- correlation path, per channel chunk ---
            for ch in range(NCHUNK):
                c0 = ch * CHW
                with tc.tile_pool(name=f"chunk{ch}", bufs=1) as pC:
                    # Wq/Wk chunk slices: [128, NC*CHW], col block k holds rows k*128..
                    Wslq = pC.tile([128, NC * CHW], f32r, tag="Wslq")
                    Wslk = pC.tile([128, NC * CHW], f32r, tag="Wslk")
                    nc.sync.dma_start(
                        Wslq.rearrange("p (k c) -> p k c", k=NC),
                        WqD[:, c0:c0 + CHW].rearrange("(k p) c -> p k c", p=128))
                    nc.sync.dma_start(
                        Wslk.rearrange("p (k c) -> p k c", k=NC),
                        WkD[:, c0:c0 + CHW].rearrange("(k p) c -> p k c", p=128))

                    # projections into combined qk tiles [t,[q|k]]
                    qk = [pC.tile([128, 2 * CHW], f32r, tag=f"qk{i}", name=f"qk{i}") for i in range(NT)]
                    with tc.tile_pool(name=f"psp{ch}", bufs=4, space=PSUM) as psp:
                        for m in range(NT):
                            for which, Wsl in ((0, Wslq), (1, Wslk)):
                                ps = psp.tile([128, CHW], f32, tag="proj")
                                for k in range(NC):
                                    nc.tensor.matmul(
                                        ps[:],
                                        xT[k][:, m * 128:(m + 1) * 128],
                                        Wsl[:, k * CHW:(k + 1) * CHW],
                                        start=(k == 0), stop=(k == NC - 1))
                                nc.vector.tensor_copy(
                                    qk[m][:, which * CHW:(which + 1) * CHW], ps[:])

                    # forward DFT + complex products
                    Pr = [pC.tile([128, CHW], f32r, tag=f"Pr{i}", name=f"Pr{i}") for i in range(NF)]
                    Pi = [pC.tile([128, CHW], f32r, tag=f"Pi{i}", name=f"Pi{i}") for i in range(NF)]
                    with (
                        tc.tile_pool(name=f"cs{ch}", bufs=3) as pCS,
                        tc.tile_pool(name=f"fs{ch}", bufs=2) as pFS,
                        tc.tile_pool(name=f"psf{ch}", bufs=2, space=PSUM) as psf,
                    ):
                        for fm in range(NF):
                            fw = 128 if fm < 8 else 1
                            pr_ps = psf.tile([128, 2 * CHW], f32, tag="cosps")
                            pi_ps = psf.tile([128, 2 * CHW], f32, tag="sinps")
                            for tk in range(NT):
                                cos_t = pCS.tile([128, 128], f32r, tag="cos")
                                sin_t = pCS.tile([128, 128], f32r, tag="sin")
                                nc.sync.dma_start(
                                    cos_t[:, :fw],
                                    CosD[tk * 128:(tk + 1) * 128, fm * 128:fm * 128 + fw])
                                nc.sync.dma_start(
                                    sin_t[:, :fw],
                                    SinD[tk * 128:(tk + 1) * 128, fm * 128:fm * 128 + fw])
                                nc.tensor.matmul(pr_ps[:fw, :], cos_t[:, :fw], qk[tk][:],
                                                 start=(tk == 0), stop=(tk == NT - 1))
                                nc.tensor.matmul(pi_ps[:fw, :], sin_t[:, :fw], qk[tk][:],
                                                 start=(tk == 0), stop=(tk == NT - 1))
                            # copies to SBUF: [Fqr|Fkr], [Gi|Ki]
                            co = pFS.tile([128, 2 * CHW], f32, tag="co")
                            si = pFS.tile([128, 2 * CHW], f32, tag="si")
                            nc.vector.tensor_copy(co[:fw, :], pr_ps[:fw, :])
                            nc.vector.tensor_copy(si[:fw, :], pi_ps[:fw, :])
                            if fm == 0:
                                nc.vector.tensor_add(
                                    co[0:1, 0:CHW], co[0:1, 0:CHW], Tbq_sb[0:1, c0:c0 + CHW])
                                nc.vector.tensor_add(
                                    co[0:1, CHW:2 * CHW], co[0:1, CHW:2 * CHW],
                                    Tbk_sb[0:1, c0:c0 + CHW])
                            # Pr = Fqr*Fkr + Gi*Ki ; Pi = Fqr*Ki - Gi*Fkr
                            u = pFS.tile([128, CHW], f32, tag="u")
                            v = pFS.tile([128, CHW], f32, tag="v")
                            fqr, fkr = co[:fw, 0:CHW], co[:fw, CHW:2 * CHW]
                            gi, ki = si[:fw, 0:CHW], si[:fw, CHW:2 * CHW]
                            nc.gpsimd.tensor_tensor(u[:fw, :], fqr, fkr, AluOpType.mult)
                            nc.vector.tensor_tensor(v[:fw, :], gi, ki, AluOpType.mult)
                            nc.vector.tensor_tensor(Pr[fm][:fw, :], u[:fw, :], v[:fw, :],
                                                    AluOpType.add)
                            u2 = pFS.tile([128, CHW], f32, tag="u2")
                            v2 = pFS.tile([128, CHW], f32, tag="v2")
                            nc.gpsimd.tensor_tensor(u2[:fw, :], fqr, ki, AluOpType.mult)
                            nc.gpsimd.tensor_tensor(v2[:fw, :], gi, fkr, AluOpType.mult)
                            nc.vector.tensor_tensor(Pi[fm][:fw, :], u2[:fw, :], v2[:fw, :],
                                                    AluOpType.subtract)

                    # inverse DFT -> corrT [c_sub, tau], then softmax+attn
                    corr = [pC.tile([128, T], f32, tag=f"corr{cs}", name=f"corr{cs}") for cs in range(CHW // 128)]
                    with (
                        tc.tile_pool(name=f"wstream{ch}", bufs=3) as pW,
                        tc.tile_pool(name=f"psi{ch}", bufs=4, space=PSUM) as psi,
                    ):
                        for tn in range(T // 512):
                            acc = [psi.tile([128, 512], f32, tag=f"inv{cs}", name=f"inv{cs}")
                                   for cs in range(CHW // 128)]
                            for fm in range(NF):
                                fh = 128 if fm < 8 else 1
                                wc_t = pW.tile([128, 512], f32r, tag="wc")
                                wn_t = pW.tile([128, 512], f32r, tag="wn")
                                nc.sync.dma_start(
                                    wc_t[:fh, :],
                                    WcD[fm * 128:fm * 128 + fh, tn * 512:(tn + 1) * 512])
                                nc.sync.dma_start(
                                    wn_t[:fh, :],
                                    WsND[fm * 128:fm * 128 + fh, tn * 512:(tn + 1) * 512])
                                for cs in range(CHW // 128):
                                    nc.tensor.matmul(
                                        acc[cs][:], Pr[fm][:fh, cs * 128:(cs + 1) * 128],
                                        wc_t[:fh, :],
                                        start=(fm == 0), stop=False)
                                    nc.tensor.matmul(
                                        acc[cs][:], Pi[fm][:fh, cs * 128:(cs + 1) * 128],
                                        wn_t[:fh, :],
                                        start=False, stop=(fm == NF - 1))
                            for cs in range(CHW // 128):
                                nc.vector.tensor_copy(
                                    corr[cs][:, tn * 512:(tn + 1) * 512], acc[cs][:])

                    with tc.tile_pool(name=f"sm{ch}", bufs=2) as pSM:
                        for cs in range(CHW // 128):
                            cc = ch * (CHW // 128) + cs
                            negmax = pSM.tile([128, 1], f32, tag="negmax")
                            nc.vector.reduce_max(negmax[:], corr[cs][:], AX, negate=True)
                            expw = pSM.tile([128, T], bf, tag="expw")
                            denom = pSM.tile([128, 1], f32, tag="denom")
                            nc.scalar.activation(expw[:], corr[cs][:], ACT.Exp,
                                                 bias=negmax[:], scale=1.0,
                                                 accum_out=denom[:])
                            recip = pSM.tile([128, 1], f32, tag="recip")
                            nc.vector.reciprocal(recip[:], denom[:])
                            att = pSM.tile([128, T], bf, tag="att")
                            nc.vector.scalar_tensor_tensor(
                                att[:], expw[:], recip[:], Vb[:],
                                op0=AluOpType.mult, op1=AluOpType.mult)
                            y1T = pSM.tile([128, T], bf, tag="y1T")
                            nc.vector.tensor_tensor(y1T[:], att[:], xT[cc].bitcast(f32)[:], AluOpType.add)
                            nc.sync.dma_start(y1T_d[cc * 128:(cc + 1) * 128, :], y1T[:])

        # --- phase 3: LN1, FFN, LN2 ---
        with tc.tile_pool(name="ffn", bufs=1) as pF:
            id16 = pF.tile([128, 128], bf, tag="id16b")
            nc.sync.dma_start(id16[:], id16D[:])
            ones16 = pF.tile([1, 128], bf, tag="ones16b")
            nc.sync.dma_start(ones16[:], ones16D[:])
            bf1T = pF.tile([128, NC], f32, tag="bf1Tb")
            nc.sync.dma_start(bf1T[:], bf1TD[:])
            bf2row = pF.tile([1, C], bf, tag="bf2rowb")
            nc.sync.dma_start(bf2row[:], bf2rowD[:])

            gb = {}
            for trivial, gD, beD, key in (
                (scalars["g1_trivial"], g1rowD, be1rowD, 1),
                (scalars["g2_trivial"], g2rowD, be2rowD, 2),
            ):
                if trivial:
                    continue
                grow = pF.tile([1, C], bf, tag=f"g{key}rowb")
                nc.sync.dma_start(grow[:], gD[:])
                berow = pF.tile([1, C], bf, tag=f"be{key}rowb")
                nc.sync.dma_start(berow[:], beD[:])
                gbt = pF.tile([128, C], bf, tag=f"g{key}bb")
                beb = pF.tile([128, C], bf, tag=f"be{key}bb")
                with tc.tile_pool(name=f"psgb{key}", bufs=2, space=PSUM) as psg:
                    for n in range(C // 512):
                        ps = psg.tile([128, 512], f32, tag="b")
                        nc.tensor.matmul(ps[:], ones16[:], grow[:, n * 512:(n + 1) * 512],
                                         start=True, stop=True)
                        nc.vector.tensor_copy(gbt[:, n * 512:(n + 1) * 512], ps[:])
                        ps2 = psg.tile([128, 512], f32, tag="b")
                        nc.tensor.matmul(ps2[:], ones16[:], berow[:, n * 512:(n + 1) * 512],
                                         start=True, stop=True)
                        nc.vector.tensor_copy(beb[:, n * 512:(n + 1) * 512], ps2[:])
                gb[key] = (gbt, beb)

            def layer_norm(tt, src_tile, dst_tile, key, pS, psS):
                """dst = LN(src) over free axis (C wide), optional affine."""
                s1 = pS.tile([128, 1], f32, tag="s1")
                nc.vector.reduce_sum(s1[:], src_tile[:], AX)
                sq = pS.tile([128, C], bf, tag="sq")
                s2 = pS.tile([128, 1], f32, tag="s2")
                nc.scalar.activation(sq[:], src_tile[:], ACT.Square, accum_out=s2[:])
                mu = pS.tile([128, 1], f32, tag="mu")
                nc.vector.tensor_scalar_mul(mu[:], s1[:], 1.0 / C)
                ex2 = pS.tile([128, 1], f32, tag="ex2")
                nc.vector.tensor_scalar_mul(ex2[:], s2[:], 1.0 / C)
                mu2 = pS.tile([128, 1], f32, tag="mu2")
                nc.vector.tensor_tensor(mu2[:], mu[:], mu[:], AluOpType.mult)
                var = pS.tile([128, 1], f32, tag="var")
                nc.vector.tensor_tensor(var[:], ex2[:], mu2[:], AluOpType.subtract)
                nc.vector.tensor_scalar_add(var[:], var[:], 1e-5)
                sd = pS.tile([128, 1], f32, tag="sd")
                nc.scalar.activation(sd[:], var[:], ACT.Sqrt)
                rstd = pS.tile([128, 1], f32, tag="rstd")
                nc.vector.reciprocal(rstd[:], sd[:])
                nc.vector.tensor_scalar(dst_tile[:], src_tile[:], mu[:], rstd[:],
                                        op0=AluOpType.subtract, op1=AluOpType.mult)
                if key in gb:
                    gbt, beb = gb[key]
                    nc.vector.tensor_tensor(dst_tile[:], dst_tile[:], gbt[:], AluOpType.mult)
                    nc.vector.tensor_tensor(dst_tile[:], dst_tile[:], beb[:], AluOpType.add)

            # rebuild y1 [t, c] from staged y1T, then LN1 -> x1
            x1 = [pF.tile([128, C], bf, tag=f"x1_{i}", name=f"x1_{i}") for i in range(NT)]
            with (
                tc.tile_pool(name="yload", bufs=4) as pY,
                tc.tile_pool(name="ps3", bufs=4, space=PSUM) as ps3,
                tc.tile_pool(name="lns", bufs=2) as pS,
            ):
                for tt in range(NT):
                    y1t = pY.tile([128, C], bf, tag="y1")
                    for cc in range(NC):
                        tmp = pY.tile([128, 128], bf, tag="ytmp")
                        nc.sync.dma_start(
                            tmp[:], y1T_d[cc * 128:(cc + 1) * 128, tt * 128:(tt + 1) * 128])
                        pt = ps3.tile([128, 128], bf, tag="tp")
                        nc.tensor.transpose(pt[:], tmp[:], id16[:])
                        nc.vector.tensor_copy(y1t[:, cc * 128:(cc + 1) * 128], pt[:])
                    layer_norm(tt, y1t, x1[tt], 1, pS, ps3)

            # x1T [c, t]
            x1T = [pF.tile([128, T], bf, tag=f"x1T{i}", name=f"x1T{i}") for i in range(NC)]
            with tc.tile_pool(name="ps4", bufs=4, space=PSUM) as ps4:
                for tt in range(NT):
                    for cc in range(NC):
                        pt = ps4.tile([128, 128], bf, tag="tp")
                        nc.tensor.transpose(pt[:], x1[tt][:, cc * 128:(cc + 1) * 128], id16[:])
                        nc.vector.tensor_copy(x1T[cc][:, tt * 128:(tt + 1) * 128], pt[:])

            # FFN
            W1t = pF.tile([128, NC * C], bf, tag="W1t")
            nc.sync.dma_start(W1t.rearrange("p (k c) -> p k c", k=NC),
                              W1D.rearrange("(k p) c -> p k c", p=128))
            W2t = pF.tile([128, NC * C], bf, tag="W2t")
            nc.sync.dma_start(W2t.rearrange("p (k c) -> p k c", k=NC),
                              W2D.rearrange("(k p) c -> p k c", p=128))

            h1 = [pF.tile([128, T], bf, tag=f"h1_{i}", name=f"h1_{i}") for i in range(NC)]
            with tc.tile_pool(name="ps5", bufs=4, space=PSUM) as ps5:
                for m in range(NC):
                    for n in range(T // 512):
                        ps = ps5.tile([128, 512], f32, tag="h1")
                        for k in range(NC):
                            nc.tensor.matmul(
                                ps[:],
                                W1t[:, k * C + m * 128:k * C + (m + 1) * 128],
                                x1T[k][:, n * 512:(n + 1) * 512],
                                start=(k == 0), stop=(k == NC - 1))
                        nc.scalar.activation(h1[m][:, n * 512:(n + 1) * 512], ps[:],
                                             ACT.Relu, bias=bf1T[:, m:m + 1], scale=1.0)

            with (
                tc.tile_pool(name="ps6", bufs=4, space=PSUM) as ps6,
                tc.tile_pool(name="y2p", bufs=3) as pY2,
                tc.tile_pool(name="lns2", bufs=2) as pS2,
            ):
                for m in range(NT):
                    y2 = pY2.tile([128, C], bf, tag="y2")
                    for n in range(C // 512):
                        ps = ps6.tile([128, 512], f32, tag="h2")
                        for k in range(NC):
                            nc.tensor.matmul(
                                ps[:],
                                h1[k][:, m * 128:(m + 1) * 128],
                                W2t[:, k * C + n * 512:k * C + (n + 1) * 512],
                                start=(k == 0), stop=False)
                        nc.tensor.matmul(ps[:], ones16[:], bf2row[:, n * 512:(n + 1) * 512],
                                         start=False, stop=True)
                        nc.vector.tensor_tensor(y2[:, n * 512:(n + 1) * 512],
                                                x1[m][:, n * 512:(n + 1) * 512], ps[:],
                                                AluOpType.add)
                    outt = pY2.tile([128, C], f32, tag="outt")
                    layer_norm(m, y2, outt, 2, pS2, ps6)
                    nc.sync.dma_start(out_d[m * 128:(m + 1) * 128, :], outt[:])

    nc.finalize()
    return nc


def kernel(**inputs):
    inputs = {k: np.asarray(v) for k, v in inputs.items()}
    x = inputs["x"].astype(np.float32)
    B = x.shape[0]
    assert x.shape == (N_CORES, T, C), f"unexpected x shape {x.shape}"

    consts, scalars = _host_constants(inputs)

    key = (scalars["g1_trivial"], scalars["g2_trivial"])
    if key not in _cache:
        _cache[key] = _build(scalars)
    nc = _cache[key]

    from concourse.bass_utils import run_bass_kernel_spmd

    in_maps = []
    for b in range(B):
        m = dict(consts)
        m["x"] = np.ascontiguousarray(x[b])
        in_maps.append(m)

    res = run_bass_kernel_spmd(nc, in_maps, list(range(N_CORES)))
    out = np.stack([res.results[b]["out"] for b in range(B)], axis=0)
    return out.astype(np.float32)



